# revision 71
# baseline (speedup 1.0000x reference)
"""Trainium2 Bass kernel for GPT2-style single attention layer.

Problem: B=4, S=2048, E=1024, H=16 heads, D=64.
  x = hidden @ W_attn + b_attn ; q,k,v = split(x)
  per head: softmax(causal(q k^T / 8) + mask) @ v
  out = merge @ W_proj + b_proj

Sharding over 8 cores: core i -> batch b = i//2, heads h0 = (i%2)*8 .. +8
(data parallel on B, tensor parallel over heads).  Each core's work is fully
local; the host sums the two partial projections per batch.

Dataflow is fully "transposed" so no on-chip transposes are ever needed:
  host feeds xT = hidden[b].T                       [E, S]
  Q^T,K^T = (Wq|Wk block)^T @ xT     -> [d, tok] per head   (W stationary)
  V       = xT_block^T @ Wv          -> [tok, d] natural    (xT stationary)
  S^T     = K^T_blk^T @ Q^T          -> [k, q]   (softmax dim on partitions)
  P^T     = exp(0.125*S^T + mask[k]) * causal01
  attn^T  = V_blk^T @ P^T            -> [d, q]  accumulated over k tiles
  sums    = ones^T @ P^T             -> [1, q]  (M=1 matmuls, PSUM-accum)
  norm    = attn^T * broadcast(1/sums)   (broadcast via K=2 ones-matmul)
  out^T   = Wp_blk^T @ attn^T        -> [col, tok]
Host transposes out^T back and sums core pairs + b_proj.

PE array tiling: the two heads of a pair run their score matmuls as
concurrent row-tiles (K=64 at partitions 0/64) and their AV matmuls as
concurrent column-tiles (M=64 at psum partitions 0/64).  Softmax
denominators are M=1 matmuls packed four to a wave into the four 32-col
strips (2 k-tiles x 2 heads), accumulated in a dedicated PSUM bank.
"""

import os
import ml_dtypes
import numpy as np

B, S, E, H, D = 4, 2048, 1024, 16, 64
NC = 8
HL = H // 2          # local heads per core
EL = HL * D          # local embedding slice = 512
P = 128              # partitions
QT = 512             # q tile width (f32 moving max)
NQT = S // QT        # 4 q tiles
NKT = S // P         # 16 k tiles
NET = E // P         # 8 e (contraction) tiles

_CACHE = {}
LAST_RESULT = None


def _build(has_bv: bool):
    from contextlib import ExitStack

    import concourse.tile as tile
    from concourse import bacc, mybir

    f32 = mybir.dt.float32
    f32r = mybir.dt.bfloat16  # matmul operand dtype (2-byte: full-rate moving operand)
    EXP = mybir.ActivationFunctionType.Exp

    nc = bacc.Bacc(
        "TRN2",
        target_bir_lowering=False,
        debug=False,
        enable_asserts=False,
        num_devices=NC,
    )

    def inp(name, shape, dt=f32):
        return nc.dram_tensor(name, shape, dt, kind="ExternalInput").ap()

    # inputs host-packed so every DMA reads contiguous 8KB per-partition
    # lines: xt[tq, p, a, c] = hidden.T[a*128+p, tq*512+c], w*[p, a, c]
    xt_d = inp("xt", [NQT, P, NET, QT], f32r)
    wq_d = inp("wq", [P, NET, EL], f32r)
    wk_d = inp("wk", [P, NET, EL], f32r)
    wv_d = inp("wv", [P, NET, EL], f32r)
    wp_d = inp("wp", [P, 4, E], f32r)
    bq_d = inp("bq", [P, 4])
    bk_d = inp("bk", [P, 4])
    bv_d = inp("bv", [P, 4])
    maskt_d = inp("maskt", [P, NKT])
    causal_d = inp("causal", [P, P], f32r)
    ones_d = inp("ones", [P, 64], f32r)
    # out[ct, qt, p, c] = outT[ct*128+p, qt*512+c]: each os tile is one
    # linear 128KB DMA write
    out_d = nc.dram_tensor("out", [NET, NQT, P, QT], f32r,
                           kind="ExternalOutput").ap()


    with tile.TileContext(nc) as tc, ExitStack() as ctx:
        const = ctx.enter_context(tc.tile_pool(name="const", bufs=1))
        big = ctx.enter_context(tc.tile_pool(name="big", bufs=1))
        wpool = ctx.enter_context(tc.tile_pool(name="wpool", bufs=1))
        xpool = ctx.enter_context(tc.tile_pool(name="xpool", bufs=1))
        ptpool = ctx.enter_context(tc.tile_pool(name="ptpool", bufs=1))
        aopool = ctx.enter_context(tc.tile_pool(name="aopool", bufs=1))
        ospool = ctx.enter_context(tc.tile_pool(name="ospool", bufs=1))
        rcpool = ctx.enter_context(tc.tile_pool(name="rcpool", bufs=1))
        aospool = ctx.enter_context(tc.tile_pool(name="aospool", bufs=1))
        psum = ctx.enter_context(tc.tile_pool(name="psum", bufs=1, space="PSUM"))

        # ---- HAM warmup ----
        # The PE clock gate sits at 4/8 (1.2 GHz) until ~3.4us of sustained
        # matmul activity.  The first ~11us of the kernel are DMA-wait (PE
        # idle), so the first real matmuls would run cold at half clock.
        # Burn the wait on dummy matmuls over a memset tile: they have no
        # DMA dependency, so they run immediately and warm the gate.
        warm_sb = const.tile([P, QT], f32r, name="warm_sb")
        nc.gpsimd.memset(warm_sb[:], 0.0)
        for g in range(3):
            wp_ps = psum.tile([P, QT], f32, name=f"warm{g}", tag="mm", bufs=2)
            for i in range(8):
                nc.tensor.matmul(wp_ps[:], warm_sb[:, 0:P], warm_sb[:],
                                 start=(i == 0), stop=(i == 7))


        # ---- persistent big buffers ----
        # Q^T / K^T: per head-pair p a [128, S] tile (partitions = 2 heads x 64 d)
        qt_tiles = [big.tile([P, S], f32r, name=f"qt{p}", tag=f"qt{p}") for p in range(4)]
        kt_tiles = [big.tile([P, S], f32r, name=f"kt{p}", tag=f"kt{p}") for p in range(4)]
        # V natural: 16 tiles [128 tok, 512 vcol]
        v_tiles = [big.tile([P, EL], f32r, name=f"v{t}", tag=f"v{t}") for t in range(NKT)]
        # W_proj: 4 partition tiles [128 elocal, 1024 col]

        # DMA emission ordered by first use: wv + xT quarter 0 gate the first
        # compute; the rest stream in behind.
        x_tiles = [[None] * NQT for _ in range(NET)]

        def load_x_quarter(tq, eng=None):
            xb = xpool.tile([P, NET * QT], f32r, name=f"xb{tq}", tag=f"xb{tq}", bufs=1)
            (eng or nc.sync).dma_start(
                xb[:].rearrange("p (a c) -> p a c", a=NET, c=QT),
                xt_d[tq],
            )
            for kt in range(NET):
                x_tiles[kt][tq] = xb[:, kt * QT:(kt + 1) * QT]

        def load_w_big(dram, label, eng=None):
            wb = wpool.tile([P, NET * EL], f32r, name=f"wb_{label}", tag=f"wb_{label}",
                            bufs=1)
            (eng or nc.sync).dma_start(
                wb[:].rearrange("p (a c) -> p a c", a=NET, c=EL),
                dram,
            )
            return [wb[:, kt * EL:(kt + 1) * EL] for kt in range(NET)]

        # startup: wv and xT quarter 0 in piecewise chunks so the first
        # V accumulation group starts as soon as the first pieces land
        wv_t = []
        xb0 = xpool.tile([P, NET * QT], f32r, name="xb0", tag="xb0", bufs=1)
        for h, (k0, k1) in enumerate(((0, 1), (1, 4), (4, 8))):
            nk = k1 - k0
            ksl = slice(k0, k1)
            wb = wpool.tile([P, nk * EL], f32r, name=f"wb_v{h}", tag=f"wb_v{h}",
                            bufs=1)
            nc.sync.dma_start(
                wb[:].rearrange("p (a c) -> p a c", a=nk, c=EL),
                wv_d[:, ksl, :],
            )
            wv_t += [wb[:, j * EL:(j + 1) * EL] for j in range(nk)]
            nc.sync.dma_start(
                xb0[:, k0 * QT:k1 * QT].rearrange(
                    "p (a c) -> p a c", a=nk, c=QT),
                xt_d[0][:, ksl, :],
            )
        for kt in range(NET):
            x_tiles[kt][0] = xb0[:, kt * QT:(kt + 1) * QT]
        wq_t = load_w_big(wq_d, "q")
        bq_t = const.tile([P, 4], f32, name="bq_t")
        nc.sync.dma_start(bq_t[:], bq_d[:])
        wk_t = load_w_big(wk_d, "k")
        bk_t = const.tile([P, 4], f32, name="bk_t")
        nc.sync.dma_start(bk_t[:], bk_d[:])
        maskt_t = const.tile([P, NKT], f32, name="maskt_t")
        nc.sync.dma_start(maskt_t[:], maskt_d[:])
        causal_t = const.tile([P, P], f32r, name="causal_t")
        nc.sync.dma_start(causal_t[:], causal_d[:])
        ones_t = const.tile([P, 64], f32r, name="ones_t")
        nc.sync.dma_start(ones_t[:], ones_d[:])
        load_x_quarter(1)
        bv_t = const.tile([P, 4], f32, name="bv_t")
        nc.sync.dma_start(bv_t[:], bv_d[:])
        wpb = wpool.tile([P, 4 * E], f32r, name="wpb", tag="wpb", bufs=1)
        nc.sync.dma_start(
            wpb[:].rearrange("p (a c) -> p a c", a=4, c=E),
            wp_d,
        )
        wp_tiles = [wpb[:, p * E:(p + 1) * E] for p in range(4)]
        for tq in range(2, NQT):
            load_x_quarter(tq)



        # ---- per-group compute units (run directly or as PE fillers) ----
        done = set()

        def v_tt(tq, tt):
            key = ("v", tq, tt)
            if key in done:
                return
            done.add(key)
            ps = psum.tile([P, EL], f32, name=f"psv{tq}_{tt}", tag="mm", bufs=2)
            for kt in range(NET):
                nc.tensor.matmul(
                    ps[:], x_tiles[kt][tq][:, tt * P:(tt + 1) * P], wv_t[kt][:],
                    start=(kt == 0), stop=(kt == NET - 1))
            nc.vector.tensor_copy(v_tiles[tq * 4 + tt][:], ps[:])

        def q_ct(tq, ct):
            key = ("q", tq, ct)
            if key in done:
                return
            done.add(key)
            ps = psum.tile([P, QT], f32, name=f"psq{tq}_{ct}", tag="mm", bufs=2)
            for kt in range(NET):
                nc.tensor.matmul(ps[:], wq_t[kt][:, ct * P:(ct + 1) * P],
                                 x_tiles[kt][tq][:],
                                 start=(kt == 0), stop=(kt == NET - 1))
            nc.vector.tensor_scalar_add(
                qt_tiles[ct][:, tq * QT:(tq + 1) * QT], ps[:], bq_t[:, ct:ct + 1])

        def k_ct(tq, ct):
            key = ("k", tq, ct)
            if key in done:
                return
            done.add(key)
            ps = psum.tile([P, QT], f32, name=f"psk{tq}_{ct}", tag="mm", bufs=2)
            for kt in range(NET):
                nc.tensor.matmul(ps[:], wk_t[kt][:, ct * P:(ct + 1) * P],
                                 x_tiles[kt][tq][:],
                                 start=(kt == 0), stop=(kt == NET - 1))
            nc.vector.tensor_scalar_add(
                kt_tiles[ct][:, tq * QT:(tq + 1) * QT], ps[:], bk_t[:, ct:ct + 1])

        def proj_ct(qt, ct, ao_tiles):
            key = ("p", qt, ct)
            if key in done:
                return
            done.add(key)
            ps = psum.tile([P, QT], f32, name=f"psp{qt}_{ct}", tag="mm", bufs=2)
            for p in range(4):
                nc.tensor.matmul(ps[:], wp_tiles[p][:, ct * P:(ct + 1) * P],
                                 ao_tiles[p][:], start=(p == 0), stop=(p == 3))
            osb = ospool.tile([P, QT], f32r, name=f"os{qt}_{ct}", tag="os", bufs=3)
            nc.vector.tensor_copy(osb[:], ps[:])
            nc.sync.dma_start(out_d[ct, qt], osb[:])

        fillers = []

        def drain_filler():
            while fillers:
                fn = fillers.pop(0)
                if fn():  # returns True if it actually emitted work
                    return


        def attention(p, qt, sga, sgb, prefetch=()):
            prefetch = list(prefetch)

            def drain_one():
                # next pair's Q/K first: they gate the next pair's scores
                while prefetch:
                    fn = prefetch.pop(0)
                    if fn():
                        return
                drain_filler()
            """Head pair p (heads 2p, 2p+1), q tile qt.

            Leaves attnout in an SBUF tile (f32->bf16) and the softmax
            denominators in rows 32*p (head a) / 32*p+1 (head b) of sg.
            Normalization happens batched per qt in normalize().

            AV runs as two concurrent column-tiles (head a -> psum rows
            0:64, head b -> 64:128).  Denominators are M=1 matmuls packed
            four per wave into the 32-col strips: for k-tile pair
            (2j, 2j+1): (a,2j)->row 0, (b,2j)->row 32, (a,2j+1)->row 64,
            (b,2j+1)->row 96 of the den psum bank."""
            kt_max = 4 * (qt + 1)
            av = psum.tile([P, QT], f32, name=f"av{p}_{qt}", tag="av", bufs=1)
            den = psum.tile([P, QT], f32, name=f"den{p}_{qt}", tag="den", bufs=1)

            def av_pair(kt, pt, off):
                first, last = kt == 0, kt == kt_max - 1
                vva = v_tiles[kt][:, (2 * p) * 64:(2 * p + 1) * 64]
                vvb = v_tiles[kt][:, (2 * p + 1) * 64:(2 * p + 2) * 64]
                nc.tensor.matmul(av[0:64, off:QT], vva, pt[:, off:QT],
                                 start=first, stop=last)
                nc.tensor.matmul(av[64:128, off:QT], vvb,
                                 pt[:, QT + off:2 * QT], start=first, stop=last)

            def den_wave(j, items):
                # items: [(kt, pt, off), (kt, pt, off)] for k-tiles 2j, 2j+1
                first, last = j == 0, j == nj - 1
                for i, (kt, pt, off) in enumerate(items):
                    # qt==0: both k-tiles share rows 0/32 (k-tile 2j+1 is
                    # diagonal with off>0 and would leave row 64/96 cols
                    # [0:off] unwritten garbage)
                    if qt == 0:
                        base = 0
                        st_f, sp_f = first and i == 0, last and i == 1
                    else:
                        base = 64 * i
                        st_f, sp_f = first, last
                    nc.tensor.matmul(den[base:base + 1, off:QT],
                                     ones_t[:, 0:1], pt[:, off:QT],
                                     start=st_f, stop=sp_f,
                                     tile_position=(0, base),
                                     skip_group_check=True)
                    nc.tensor.matmul(den[base + 32:base + 33, off:QT],
                                     ones_t[:, 0:1], pt[:, QT + off:2 * QT],
                                     start=st_f, stop=sp_f,
                                     tile_position=(0, base + 32),
                                     skip_group_check=True)

            def score_tile(kt):
                # diagonal tiles: only q columns >= off are unmasked
                diag = kt >= qt * 4
                off = (kt - qt * 4) * P if diag else 0
                kl = slice(kt * P, (kt + 1) * P)
                qv = slice(qt * QT + off, (qt + 1) * QT)
                st = psum.tile([P, 2 * QT], f32, name=f"st{p}_{qt}_{kt}",
                               tag="st", bufs=2)
                nc.tensor.matmul(st[:, off:QT], kt_tiles[p][0:64, kl],
                                 qt_tiles[p][0:64, qv])
                nc.tensor.matmul(st[:, QT + off:2 * QT], kt_tiles[p][64:128, kl],
                                 qt_tiles[p][64:128, qv])
                pt = ptpool.tile([P, 2 * QT], f32r, name=f"pt{p}_{qt}_{kt}",
                                 tag="pt", bufs=5)
                bias = maskt_t[:, kt:kt + 1]
                if not diag or off == 0:
                    nc.scalar.activation(pt[:], st[:], EXP, bias=bias, scale=0.125)
                else:
                    stv = st[:].rearrange("p (h q) -> p h q", h=2, q=QT)[:, :, off:QT]
                    ptv = pt[:].rearrange("p (h q) -> p h q", h=2, q=QT)[:, :, off:QT]
                    nc.scalar.activation(ptv, stv, EXP, bias=bias, scale=0.125)
                if diag:
                    # triangular band at the leading 128 valid columns
                    nc.vector.tensor_mul(pt[:, off:off + P], pt[:, off:off + P],
                                         causal_t[:])
                    nc.vector.tensor_mul(pt[:, QT + off:QT + off + P],
                                         pt[:, QT + off:QT + off + P], causal_t[:])
                return (kt, pt, off)

            nj = kt_max // 2
            pending = None
            for j in range(nj):
                items = [score_tile(2 * j), score_tile(2 * j + 1)]
                if pending is not None:
                    jp, prev = pending
                    # den wave emitted after both AV pairs: all four den
                    # matmuls are then simultaneously ready (both pt tiles
                    # exist), so nothing can split the 4-strip wave
                    av_pair(*prev[0])
                    av_pair(*prev[1])
                    den_wave(jp, prev)
                    drain_one()
                elif prefetch:
                    drain_one()
                pending = (j, items)
            jp, prev = pending
            av_pair(*prev[0])
            av_pair(*prev[1])
            den_wave(jp, prev)

            # drain PSUM immediately so the next pair's AV can start
            aos = aospool.tile([P, QT], f32r, name=f"aos{p}_{qt}",
                               tag=f"aos{p}", bufs=2)
            row = 32 * p
            nc.vector.tensor_copy(aos[:], av[:])
            if qt == 0:
                nc.vector.tensor_copy(sga[row:row + 1, :], den[0:1, :])
                nc.vector.tensor_copy(sgb[row:row + 1, :], den[32:33, :])
            else:
                # DVE can read only one PSUM operand per instruction: stage
                # the odd-k-tile partials (rows 64/96) through SBUF first
                dcp = rcpool.tile([33, QT], f32, name=f"dcp{p}_{qt}",
                                  tag="dcp", bufs=2)
                nc.vector.tensor_copy(dcp[:], den[64:97, :])
                nc.vector.tensor_add(sga[row:row + 1, :], den[0:1, :],
                                     dcp[0:1, :])
                nc.vector.tensor_add(sgb[row:row + 1, :], den[32:33, :],
                                     dcp[32:33, :])
            return aos

        def normalize(qt, sga, sgb, aos_tiles):
            """Batched softmax normalization for all 4 pairs of one q tile."""
            rcf = rcpool.tile([97, QT], f32, name=f"rcf{qt}", tag="rcf", bufs=1)
            rcg = rcpool.tile([97, QT], f32, name=f"rcg{qt}", tag="rcg", bufs=1)
            nc.vector.reciprocal_approx_fast(rcf[:], sga[:])
            nc.vector.reciprocal_approx_fast(rcg[:], sgb[:])
            rca = rcpool.tile([97, QT], f32r, name=f"rca{qt}", tag="rca", bufs=1)
            rcb = rcpool.tile([97, QT], f32r, name=f"rcb{qt}", tag="rcb", bufs=1)
            nc.vector.tensor_copy(rca[:], rcf[:])
            nc.vector.tensor_copy(rcb[:], rcg[:])
            ao_tiles = []
            for p in range(4):
                row = 32 * p
                ao = aopool.tile([P, QT], f32r, name=f"ao{p}_{qt}",
                                 tag=f"ao{p}", bufs=2)
                # broadcast recip_a to psum rows 0:64 and recip_b to 64:128
                # as a concurrent column-tiled matmul pair
                rb = psum.tile([P, QT], f32, name=f"rb{p}_{qt}", tag="mm",
                               bufs=2)
                nc.tensor.matmul(rb[0:64, :], ones_t[row:row + 1, 0:64],
                                 rca[row:row + 1, :], tile_position=(row, 0))
                nc.tensor.matmul(rb[64:128, :], ones_t[row:row + 1, 0:64],
                                 rcb[row:row + 1, :], tile_position=(row, 64))
                nc.vector.tensor_mul(ao[:], rb[:], aos_tiles[p][:])
                if has_bv:
                    nc.vector.tensor_scalar_add(ao[:], ao[:], bv_t[:, p:p + 1])
                ao_tiles.append(ao)
            return ao_tiles


        # ============ filler-queue main schedule ============
        # Attention k-loops are ACT(exp)-paced; PE idle slots are filled with
        # independent matmul groups: next quarter's V/Q/K and deferred proj.
        def mkfiller(fn, *args):
            def run():
                before = len(done)
                fn(*args)
                return len(done) != before
            return run

        pending_np = None
        deferred_proj = None
        late_proj = []
        for tq in range(NQT):
            if tq == NQT - 1:
                # half of proj(qt1) was held back: release it into the last
                # quarter, whose long causal attention otherwise drains the
                # filler queue dry
                fillers.extend(late_proj)
                late_proj = []
            # mandatory prelude: V + first pair's Q/K; later pairs become
            # fillers drained (or ensured) just in time
            for tt in range(4):
                v_tt(tq, tt)
            q_ct(tq, 0)
            k_ct(tq, 0)
            for ct in range(1, 4):
                fillers.append(mkfiller(q_ct, tq, ct))
                fillers.append(mkfiller(k_ct, tq, ct))
            # queue next quarter's V/Q/K as fillers
            if tq + 1 < NQT:
                for tt in range(4):
                    fillers.append(mkfiller(v_tt, tq + 1, tt))
                for ct in range(4):
                    fillers.append(mkfiller(q_ct, tq + 1, ct))
                    fillers.append(mkfiller(k_ct, tq + 1, ct))
            sga = rcpool.tile([97, QT], f32, name=f"sga{tq}", tag="sga", bufs=2)
            sgb = rcpool.tile([97, QT], f32, name=f"sgb{tq}", tag="sgb", bufs=2)
            aos_tiles = []
            for p in range(4):
                q_ct(tq, p)
                k_ct(tq, p)
                if p < 3:
                    pf = [mkfiller(q_ct, tq, p + 1), mkfiller(k_ct, tq, p + 1)]
                elif tq + 1 < NQT:
                    pf = [mkfiller(q_ct, tq + 1, 0), mkfiller(k_ct, tq + 1, 0)]
                else:
                    pf = []
                aos_tiles.append(attention(p, tq, sga, sgb, prefetch=pf))
                if p == 1 and pending_np is not None:
                    qt_prev, ao_prev = pending_np[0], normalize(*pending_np)
                    # enqueue proj late (half at p==1, half at p==3) so the
                    # filler queue is not exhausted before the last quarter
                    for ct in range(NET // 2):
                        fillers.append(mkfiller(proj_ct, qt_prev, ct, ao_prev))
                    deferred_proj = (qt_prev, ao_prev)
                    pending_np = None
                if p == 3 and deferred_proj is not None:
                    qt_prev, ao_prev = deferred_proj
                    dst = late_proj if qt_prev == 1 else fillers
                    for ct in range(NET // 2, NET):
                        dst.append(mkfiller(proj_ct, qt_prev, ct, ao_prev))
                    deferred_proj = None
            pending_np = (tq, sga, sgb, aos_tiles)
        # keepalive: the normalize(qt3) DVE chain leaves the PE idle ~3.5us,
        # long enough for the HAM clock gate to drop to 4/8 — which would run
        # the whole proj tail at half clock.  Dummy matmuls reading the last
        # pair's attention output become ready exactly in that hole.
        ka = pending_np[3][3]
        for g in range(3):
            kp = psum.tile([P, QT], f32, name=f"keep{g}", tag="mm", bufs=2)
            for i in range(4):
                nc.tensor.matmul(kp[:], warm_sb[:, 0:P], ka[:],
                                 start=(i == 0), stop=(i == 3))
        # final: leftover fillers, then last quarter's normalize + proj
        while fillers:
            fillers.pop(0)()
        qt_last, ao_last = pending_np[0], normalize(*pending_np)
        for ct in range(NET):
            proj_ct(qt_last, ct, ao_last)

    nc.compile()
    return nc


def _causal_tiles():
    """[128, 128] lower-triangular 0/1 band mask (dq >= dk)."""
    dk = np.arange(P)[:, None]
    dq = np.arange(P)[None, :]
    return np.ascontiguousarray((dq >= dk).astype(np.float32))


def kernel(hidden_state, attention_mask, W_attn, b_attn, W_proj, b_proj):
    global LAST_RESULT
    hs = np.asarray(hidden_state, np.float32)
    am = np.asarray(attention_mask, np.float32).reshape(B, S)
    wa = np.asarray(W_attn, np.float32)
    ba = np.asarray(b_attn, np.float32)
    wpr = np.asarray(W_proj, np.float32)
    bp = np.asarray(b_proj, np.float32)

    has_bv = bool(np.any(ba[2 * E:3 * E] != 0.0))
    key = ("k", has_bv)
    if key not in _CACHE:
        _CACHE[key] = _build(has_bv)
    nc = _CACHE[key]

    bf16 = ml_dtypes.bfloat16
    causal = _causal_tiles().astype(bf16)
    def pack_w(w):
        # [E, EL] -> [P, NET, EL]: w_packed[p, a, c] = w[a*128+p, c]
        return np.ascontiguousarray(
            w.reshape(NET, P, -1).transpose(1, 0, 2)).astype(bf16)

    in_maps = []
    for core in range(NC):
        b = core // 2
        c0 = (core % 2) * EL
        xt = hs[b].T  # [E, S]
        in_maps.append({
            # [NQT, P, NET, QT]: xt_packed[tq, p, a, c] = xt[a*128+p, tq*512+c]
            "xt": np.ascontiguousarray(
                xt.reshape(NET, P, NQT, QT).transpose(2, 1, 0, 3)).astype(bf16),
            "wq": pack_w(wa[:, c0:c0 + EL]),
            "wk": pack_w(wa[:, E + c0:E + c0 + EL]),
            "wv": pack_w(wa[:, 2 * E + c0:2 * E + c0 + EL]),
            # [P, 4, E]: wp_packed[p, a, c] = wp_slice[a*128+p, c]
            "wp": np.ascontiguousarray(
                wpr[c0:c0 + EL, :].reshape(4, P, E).transpose(1, 0, 2)
            ).astype(bf16),
            "bq": np.ascontiguousarray(ba[c0:c0 + EL].reshape(4, P).T),
            "bk": np.ascontiguousarray(ba[E + c0:E + c0 + EL].reshape(4, P).T),
            "bv": np.ascontiguousarray(ba[2 * E + c0:2 * E + c0 + EL].reshape(4, P).T),
            "maskt": np.ascontiguousarray(am[b].reshape(NKT, P).T),
            "causal": causal,
            "ones": np.ones((P, 64), bf16),
        })

    from concourse.bass_utils import run_bass_kernel_spmd

    trace = os.environ.get("KERNEL_TRACE", "") == "1"
    res = run_bass_kernel_spmd(nc, in_maps, core_ids=list(range(NC)), trace=trace)
    LAST_RESULT = res

    def unpack_out(o):
        # [NET, NQT, P, QT] -> [S, E]: out[qt*512+c, ct*128+p]
        return o.transpose(1, 3, 0, 2).reshape(S, E)

    full = np.empty((B, S, E), np.float32)
    for b in range(B):
        full[b] = unpack_out(res.results[2 * b]["out"]).astype(np.float32)
        full[b] += unpack_out(res.results[2 * b + 1]["out"]).astype(np.float32)
        full[b] += bp
    return full

